# revision 1
# baseline (speedup 1.0000x reference)
"""Trainium2 Bass kernel for AttentionReadoutAtom (global-softmax segment reduce).

Math:  scores = x @ w + b ; attn = softmax(scores over all N) ;
       out[s] = sum_{i: label_i = s} attn_i * x_i          -> [50000, 128]

Softmax is shift/scale invariant: exp(score) without max-subtraction is safe
here (scores ~ N(0,1)), and the bias b cancels between numerator and
denominator.  Using xw = x * w (host-side sharding layout prep):

    out[s, d] = sum_{i in s} e_i * xw_i[d] / (w[d] * Z),   Z = sum_i e_i

Sharding (host, inside kernel()):
  * Sort rows by segment label; greedily pack whole segments into blocks of
    1024 rows (8 tiles of 128 rows) covering <= 128 distinct segments each;
    pad each block to 1024 rows with zero rows.  Every segment lives in
    exactly one block -> no cross-core combination of outputs is needed;
    the only global quantity is the softmax denominator Z, reduced on the
    host from per-core partial e sums (the hint's denominator all-reduce).
  * Blocks are dealt contiguously to 8 cores, padded to equal count B.
  * xw is shipped bf16 (FAST) or as a bf16 hi/lo pair (SPLIT, default),
    pre-arranged [B, half, 128, 1024] so every DMA is a contiguous 2KB/row
    super-tile.

Device per row-tile t of a block (Tile framework schedules all engines):
  * score[p] = sum_d xw[t*128+p, d]     (DVE tensor_scalar accum_out, with
               some row-tiles' score op placed on ScalarE to balance engines)
  * e = exp(score)                      (ScalarE, one op per 8-block chunk;
               e chunk is DMA'd out for the host-side Z reduction)
  * Me[p, s] = (iota[s] == lab_rel[p]) * e[p]   (one DVE tensor_scalar,
               dual-ALU: is_equal then mult with two [P,1] operands; the
               lo-half Me_l = Me_h * (e_lo/e_hi) uses the cheaper single-op
               form, 254 vs 313 ns measured)
  * psum[s, d] += Me^T @ xw_tile        (TensorE, PSUM accumulation over the
               block; SPLIT mode issues Mh@xh + Mh@xl + Ml@xh, which
               reconstructs the f32 product to ~4e-5 because bf16*bf16
               products are exact in the PE's f32 accumulation)
  * evict psum -> SBUF -> DRAM          (VectorE copy + DMA)

Host epilogue: scatter per-block rows to the full [50000, 128] output and
apply the scalar normalization out / (w[d] * Z).

Measured on 8 trn2 NeuronCores (NTFF profile, max across cores):
  MODE=split (default): HW exec ~326 us, scale-relative absmax err ~9.5e-6
                        (error floor set by the ScalarE exp LUT, ~1e-5 rel)
  MODE=fast:            HW exec ~252 us, scale-relative absmax err ~4.1e-3
Memory roofline (37 MB/core at ~360 GB/s) is ~103 us; the gap is per-
instruction overhead: the per-row-tile ops (DVE TensorScalarPtr ~310 ns,
ScalarE ACTIVATE+READ_ACC ~750 ns, PE LDWEIGHTS+MATMUL ~400 ns) dominate,
and every 128-row tile fundamentally needs one one-hot build + one score
reduction + matmul(s).
"""

import os
import numpy as np
import ml_dtypes

# ---------------------------------------------------------------- constants
N = 500000
D = 128
NUM_SEGMENTS = 50000
N_CORES = 8
P = 128
TPB = 8                   # row tiles per block
ROWS_PER_BLOCK = TPB * P  # 1024
MAX_SEGS_PER_BLOCK = 128
CHUNK_BLOCKS = 4          # blocks per e/lab chunk (32 row tiles)

MODE = os.environ.get("ATTN_KERNEL_MODE", "split")  # "split" | "fast"
# number of leading blocks per chunk whose score op runs on DVE (rest: ScalarE)
DVE_SCORE_MOD = {"split": 16, "fast": 2}  # every k-th row-tile's score on DVE (must divide TPB)

_COMPILED = {}


# ---------------------------------------------------------------- device code
def _build_kernel(B, mode):
    import concourse.bacc as bacc
    import concourse.mybir as mybir
    from concourse.tile import TileContext

    f32 = mybir.dt.float32
    bf16 = mybir.dt.bfloat16
    Alu = mybir.AluOpType
    Act = mybir.ActivationFunctionType

    nsplit = 2 if mode == "split" else 1
    NT = B * TPB
    NCHUNK = (B + CHUNK_BLOCKS - 1) // CHUNK_BLOCKS
    CC = CHUNK_BLOCKS * TPB          # score/e columns per chunk
    dve_mod = DVE_SCORE_MOD[mode]

    nc = bacc.Bacc("TRN2", target_bir_lowering=False, debug=False,
                   num_devices=N_CORES)

    xw_d = nc.dram_tensor("xw", [B, P, TPB * nsplit * P], bf16,
                          kind="ExternalInput")
    lab_d = nc.dram_tensor("lab", [NCHUNK, P, CC], f32, kind="ExternalInput")
    out_d = nc.dram_tensor("out", [B, P, P], f32, kind="ExternalOutput")
    z_d = nc.dram_tensor("zpart", [NCHUNK, P, CC], f32, kind="ExternalOutput")

    with TileContext(nc) as tc:
        with tc.tile_pool(name="const", bufs=1) as cpool, \
             tc.tile_pool(name="xwp", bufs=3 * CHUNK_BLOCKS) as xwp, \
             tc.tile_pool(name="labp", bufs=3) as labp, \
             tc.tile_pool(name="scp", bufs=4) as scp, \
             tc.tile_pool(name="mep", bufs=16) as mep, \
             tc.tile_pool(name="evp", bufs=4) as evp, \
             tc.tile_pool(name="psum", bufs=8, space="PSUM") as psp:

            iota_i = cpool.tile([P, P], mybir.dt.int32)
            nc.gpsimd.iota(iota_i[:], pattern=[[1, P]], base=0,
                           channel_multiplier=0)
            iota_b = cpool.tile([P, P], bf16)
            nc.vector.tensor_copy(iota_b[:], iota_i[:])

            for ch in range(NCHUNK):
                blocks = list(range(ch * CHUNK_BLOCKS,
                                    min((ch + 1) * CHUNK_BLOCKS, B)))
                nb = len(blocks)
                ntile = nb * TPB

                lab_t = labp.tile([P, CC], f32, tag="lab")
                nc.sync.dma_start(lab_t[:, :ntile], lab_d.ap()[ch, :, :ntile])

                sc_dve = scp.tile([P, CC], f32, tag="sc_dve")
                sc_act = scp.tile([P, CC], f32, tag="sc_act")
                e_t = scp.tile([P, CC], f32, tag="e")
                if mode == "split":
                    ehib_t = scp.tile([P, CC], bf16, tag="ehib")
                    elo_t = scp.tile([P, CC], f32, tag="elo")
                    rec_t = scp.tile([P, CC], f32, tag="rec")
                    r_t = scp.tile([P, CC], f32, tag="r")
                junk_d = scp.tile([P, nsplit * P], bf16, tag="junk_d")
                junk_a = scp.tile([P, nsplit * P], bf16, tag="junk_a")

                xw_tiles = []
                for bi, b in enumerate(blocks):
                    xw_t = xwp.tile([P, nsplit * TPB * P], bf16, tag="xw")
                    nc.sync.dma_start(xw_t[:], xw_d.ap()[b, :, :])
                    xw_tiles.append(xw_t)
                    W = nsplit * P
                    for t in range(TPB):
                        col = bi * TPB + t
                        src = xw_t[:, t * W:(t + 1) * W]   # [P, nsplit*128] 2D
                        if col % dve_mod == 0 and ntile >= dve_mod:
                            nc.vector.tensor_scalar(
                                out=junk_d[:], in0=src,
                                scalar1=1.0, scalar2=0.0,
                                op0=Alu.mult, op1=Alu.add,
                                accum_out=sc_dve[:, col:col + 1])
                        else:
                            nc.scalar.activation(
                                out=junk_a[:], in_=src, func=Act.Copy,
                                accum_out=sc_act[:, col:col + 1])

                # e = exp(score): strided views select each engine's columns
                sc3d = sc_dve[:].rearrange("p (g k) -> p g k", k=dve_mod)
                sa3d = sc_act[:].rearrange("p (g k) -> p g k", k=dve_mod)
                e3d = e_t[:].rearrange("p (g k) -> p g k", k=dve_mod)
                ng = ntile // dve_mod
                nc.scalar.activation(out=e3d[:, :ng, 0:1],
                                     in_=sc3d[:, :ng, 0:1], func=Act.Exp)
                nc.scalar.activation(out=e3d[:, :ng, 1:dve_mod],
                                     in_=sa3d[:, :ng, 1:dve_mod], func=Act.Exp)
                # ship e for the host-side Z reduction (pads in the last
                # group of a short chunk were never written: zero them via
                # host-side masking instead -> here just DMA what exists)
                nc.sync.dma_start(z_d.ap()[ch, :, :ntile], e_t[:, :ntile])
                if mode == "split":
                    nc.vector.tensor_copy(ehib_t[:, :ntile], e_t[:, :ntile])
                    nc.vector.tensor_tensor(
                        out=elo_t[:, :ntile], in0=e_t[:, :ntile],
                        in1=ehib_t[:, :ntile], op=Alu.subtract)
                    # r = e_lo / e_hi so Me_l can be built from Me_h with a
                    # cheap single-op tensor_scalar (measured 254 vs 313 ns)
                    nc.vector.reciprocal(rec_t[:, :ntile], ehib_t[:, :ntile])
                    nc.vector.tensor_tensor(
                        out=r_t[:, :ntile], in0=elo_t[:, :ntile],
                        in1=rec_t[:, :ntile], op=Alu.mult)

                for bi, b in enumerate(blocks):
                    xw_t = xw_tiles[bi]
                    ps = psp.tile([P, P], f32, tag="acc")
                    n_mm = 3 * TPB if mode == "split" else TPB
                    mm = 0
                    for t in range(TPB):
                        col = bi * TPB + t
                        me_h = mep.tile([P, P], bf16, tag="meh")
                        nc.vector.tensor_scalar(
                            out=me_h[:], in0=iota_b[:],
                            scalar1=lab_t[:, col:col + 1],
                            scalar2=e_t[:, col:col + 1],
                            op0=Alu.is_equal, op1=Alu.mult)
                        xh = xw_t[:, t * nsplit * P:(t * nsplit + 1) * P]
                        nc.tensor.matmul(ps[:], lhsT=me_h[:], rhs=xh,
                                         start=(mm == 0),
                                         stop=(mm == n_mm - 1))
                        mm += 1
                        if mode == "split":
                            xl = xw_t[:, (t * nsplit + 1) * P:(t * nsplit + 2) * P]
                            nc.tensor.matmul(ps[:], lhsT=me_h[:], rhs=xl,
                                             start=False,
                                             stop=(mm == n_mm - 1))
                            mm += 1
                            me_l = mep.tile([P, P], bf16, tag="mel")
                            nc.vector.tensor_scalar(
                                out=me_l[:], in0=me_h[:],
                                scalar1=r_t[:, col:col + 1],
                                scalar2=None, op0=Alu.mult)
                            nc.tensor.matmul(ps[:], lhsT=me_l[:], rhs=xh,
                                             start=False,
                                             stop=(mm == n_mm - 1))
                            mm += 1
                    ev = evp.tile([P, P], f32, tag="ev")
                    nc.vector.tensor_copy(ev[:], ps[:])
                    nc.sync.dma_start(out_d.ap()[b, :, :], ev[:])

    nc.compile()
    return nc


# ---------------------------------------------------------------- host side
def _pack_blocks(counts):
    blocks = []
    s, nseg = 0, len(counts)
    while s < nseg:
        rows, s0 = 0, s
        while s < nseg and s - s0 < MAX_SEGS_PER_BLOCK:
            c = counts[s]
            if rows + c > ROWS_PER_BLOCK:
                break
            rows += int(c)
            s += 1
        assert s > s0, f"segment {s0} with {counts[s0]} rows exceeds a block"
        blocks.append((s0, s, rows))
    return blocks


def _numpy_fallback(x, labels, w, b):
    scores = x.astype(np.float64) @ w.astype(np.float64) + float(b)
    scores -= scores.max()
    e = np.exp(scores)
    a = e / e.sum()
    out = np.zeros((NUM_SEGMENTS, x.shape[1]), np.float64)
    np.add.at(out, labels, x * a[:, None])
    return out.astype(np.float32)


def kernel(x, monomer_labels_i, attn_w, attn_b):
    from concourse import bass_utils

    x = np.ascontiguousarray(np.asarray(x, dtype=np.float32))
    labels = np.asarray(monomer_labels_i).astype(np.int64)
    w = np.asarray(attn_w, dtype=np.float32)
    b = np.float32(np.asarray(attn_b))

    if np.abs(w).min() < 1e-30 or np.bincount(
            labels, minlength=NUM_SEGMENTS).max() > ROWS_PER_BLOCK:
        return _numpy_fallback(x, labels, w, b)

    order = np.argsort(labels, kind="stable")
    labels_s = labels[order]
    counts = np.bincount(labels, minlength=NUM_SEGMENTS)
    blocks = _pack_blocks(counts)
    nblocks = len(blocks)
    B = (nblocks + N_CORES - 1) // N_CORES
    NCHUNK = (B + CHUNK_BLOCKS - 1) // CHUNK_BLOCKS
    CC = CHUNK_BLOCKS * TPB
    seg_row_start = np.zeros(NUM_SEGMENTS + 1, np.int64)
    np.cumsum(counts, out=seg_row_start[1:])

    nsplit = 2 if MODE == "split" else 1
    xw = x[order] * w[None, :]
    xw_hi = xw.astype(ml_dtypes.bfloat16)
    if MODE == "split":
        xw_lo = (xw - xw_hi.astype(np.float32)).astype(ml_dtypes.bfloat16)

    in_maps = []
    meta = []
    n_pad_rows = 0
    for c in range(N_CORES):
        xw_dev = np.zeros((B, P, TPB, nsplit, P), ml_dtypes.bfloat16)
        lab_dev = np.full((NCHUNK, P, CC), 127.0, np.float32)
        meta_c = []
        for bi in range(B):
            gi = c * B + bi
            if gi >= nblocks:
                meta_c.append(None)
                n_pad_rows += ROWS_PER_BLOCK
                continue
            s0, s1, rows = blocks[gi]
            r0 = seg_row_start[s0]
            ch, pos = divmod(bi, CHUNK_BLOCKS)

            def pack(src_rows):
                full = np.zeros((ROWS_PER_BLOCK, D), src_rows.dtype)
                full[:rows] = src_rows
                return full.reshape(TPB, P, D).transpose(1, 0, 2).reshape(
                    P, TPB * P)

            xw_dev[bi, :, :, 0, :] = pack(xw_hi[r0:r0 + rows]).reshape(
                P, TPB, D)
            if MODE == "split":
                xw_dev[bi, :, :, 1, :] = pack(xw_lo[r0:r0 + rows]).reshape(
                    P, TPB, D)
            fl = np.full(ROWS_PER_BLOCK, 127.0, np.float32)
            fl[:rows] = (labels_s[r0:r0 + rows] - s0).astype(np.float32)
            lab_dev[ch, :, pos * TPB:(pos + 1) * TPB] = \
                fl.reshape(TPB, P).transpose(1, 0)
            n_pad_rows += ROWS_PER_BLOCK - rows
            meta_c.append((int(s0), int(s1)))
        meta.append(meta_c)
        in_maps.append({"xw": xw_dev.reshape(B, P, TPB * nsplit * P),
                        "lab": lab_dev})

    key = (B, MODE)
    if key not in _COMPILED:
        _COMPILED[key] = _build_kernel(B, MODE)
    nc = _COMPILED[key]

    res = bass_utils.run_bass_kernel_spmd(nc, in_maps,
                                          core_ids=list(range(N_CORES)))

    # ---- gather / unshard
    Z = 0.0
    out = np.zeros((NUM_SEGMENTS, D), np.float32)
    for c in range(N_CORES):
        r = res.results[c]
        zp = r["zpart"]
        for ch in range(NCHUNK):
            ntile = (min((ch + 1) * CHUNK_BLOCKS, B) - ch * CHUNK_BLOCKS) * TPB
            Z += float(zp[ch, :, :ntile].astype(np.float64).sum())
        out_dev = r["out"]
        for bi in range(B):
            m = meta[c][bi]
            if m is None:
                continue
            s0, s1 = m
            out[s0:s1] = out_dev[bi, :s1 - s0, :]
    # pad rows have xw == 0 -> score 0 -> e = exp(0) = 1 each
    Z -= float(n_pad_rows)
    out /= (w[None, :] * np.float32(Z))
    return out.astype(np.float32)


if __name__ == "__main__":
    from ref_io import get
    inputs, expected = get()
    out = kernel(**inputs)
    err = np.abs(out - expected)
    print("absmax err:", err.max(), "scale-rel:",
          err.max() / np.abs(expected).max())



# revision 2
# speedup vs baseline: 1.8253x; 1.8253x over previous
"""Trainium2 Bass kernel for AttentionReadoutAtom (global-softmax segment reduce).

Math:  scores = x @ w + b ; attn = softmax(scores over all N) ;
       out[s] = sum_{i: label_i = s} attn_i * x_i          -> [50000, 128]

Softmax is shift/scale invariant, so exp(score) without max-subtraction is
safe (scores ~ N(0,1)) and the bias b cancels.  Using xw = x * w:

    out[s, d] = sum_{i in s} e_i * xw_i[d] / (w[d] * Z),   Z = sum_i e_i

Sharding (host, inside kernel()): sort rows by segment label, greedily pack
whole segments into blocks of 1024 rows (8 row-tiles of 128) covering <=128
distinct segments; blocks are dealt contiguously to 8 cores.  Every segment
lives in exactly one block, so no cross-core combine is needed; the only
global quantity is Z, reduced on the host (the hint's denominator
all-reduce).

Device, per block (Tile framework schedules all engines):
  * score'[p, t] = sum_w xw_aug[p, t, w]  -- ONE grouped DVE tensor_reduce
    per block over a [128, 8, 130] view.  Column 128 of each tile is 1.0,
    col 129 is 0.0, so score' = score + 1: a constant softmax shift that
    cancels in the normalization.
  * me[p, s] = e'_p * onehot(lab_p)[s], built one of two ways per row-tile
    to balance engines:
      - ScalarE tiles: me = Exp(logmask + score') in a single fused
        ACTIVATE: logmask is a host-shipped fp8 (0 / -96) [128, 128] tile,
        score' rides the per-partition bias port.  exp(-96+s) == 0.
      - DVE tiles: me = (iota == lab) * e' in one dual-ALU tensor_scalar
        (e' comes from a small batched exp of the DVE score columns).
  * psum[s, w] += me^T @ xw_aug_tile  (TensorE, PSUM accum over the block).
    Column 128 of the result is sum of e' per segment (pad rows have
    me == 0 everywhere, so they are excluded automatically) -> Z on host.
  * evict psum -> SBUF -> DRAM.

Host epilogue: Z = sum of column-128, scatter block rows to the full
output, divide by w[d] * Z.

Baseline (per-row-tile score ops + hi/lo split matmuls): 324 us HW.
This version: one reduce + ~8 engine-balanced one-hot ops + 8 matmuls per
block; DMA ~25 MB/core.
"""

import os
import numpy as np
import ml_dtypes

# ---------------------------------------------------------------- constants
N = 500000
D = 128
NUM_SEGMENTS = 50000
N_CORES = 8
P = 128
TPB = 8                   # row tiles per block
ROWS_PER_BLOCK = TPB * P  # 1024
MAX_SEGS_PER_BLOCK = 128
W = 130                   # cols per tile in xw_aug: 128 xw + 1.0 + 0.0
LABW = 4                  # label slots per block (max DVE tiles)
CHUNK_BLOCKS = 4          # blocks per chunk (batched exp granularity)

# ScalarE tiles per block, by local block parity.  8 - nsc tiles go to DVE.
_pat = os.environ.get("ATTN_NSC_PATTERN", "5,4")
NSC_PATTERN = tuple(int(v) for v in _pat.split(","))
MAXNSC = max(NSC_PATTERN)
assert all(8 - s <= LABW for s in NSC_PATTERN)

_COMPILED = {}


def _nsc(bi):
    return NSC_PATTERN[bi % len(NSC_PATTERN)]


# ---------------------------------------------------------------- device code
def _build_kernel(B):
    import concourse.bacc as bacc
    import concourse.mybir as mybir
    from concourse.tile import TileContext

    f32 = mybir.dt.float32
    bf16 = mybir.dt.bfloat16
    f8 = mybir.dt.float8e4
    Alu = mybir.AluOpType
    Act = mybir.ActivationFunctionType
    Ax = mybir.AxisListType

    nc = bacc.Bacc("TRN2", target_bir_lowering=False, debug=False,
                   num_devices=N_CORES)

    xw_d = nc.dram_tensor("xw", [B, P, TPB * W], bf16, kind="ExternalInput")
    mk_d = nc.dram_tensor("mk", [B, P, MAXNSC * P], f8, kind="ExternalInput")
    lab_d = nc.dram_tensor("lab", [P, B * LABW], f32, kind="ExternalInput")
    out_d = nc.dram_tensor("out", [B, P, P + 1], f32, kind="ExternalOutput")

    with TileContext(nc) as tc:
        with tc.tile_pool(name="const", bufs=1) as cpool, \
             tc.tile_pool(name="xwp", bufs=2 * CHUNK_BLOCKS) as xwp, \
             tc.tile_pool(name="mkp", bufs=2 * CHUNK_BLOCKS) as mkp, \
             tc.tile_pool(name="scp", bufs=3) as scp, \
             tc.tile_pool(name="mep", bufs=12) as mep, \
             tc.tile_pool(name="evp", bufs=6) as evp, \
             tc.tile_pool(name="psum", bufs=8, space="PSUM") as psp:

            iota_i = cpool.tile([P, P], mybir.dt.int32)
            nc.gpsimd.iota(iota_i[:], pattern=[[1, P]], base=0,
                           channel_multiplier=0)
            iota_b = cpool.tile([P, P], bf16)
            nc.vector.tensor_copy(iota_b[:], iota_i[:])

            lab_all = cpool.tile([P, B * LABW], f32)
            nc.sync.dma_start(lab_all[:], lab_d.ap()[:, :])

            NCHUNK = (B + CHUNK_BLOCKS - 1) // CHUNK_BLOCKS
            for ch in range(NCHUNK):
                blocks = list(range(ch * CHUNK_BLOCKS,
                                    min((ch + 1) * CHUNK_BLOCKS, B)))
                nb = len(blocks)

                sc_t = scp.tile([P, nb * TPB], f32, tag="sc")
                e_t = scp.tile([P, nb * LABW], f32, tag="e")

                xw_tiles, mk_tiles = [], []
                for bi, b in enumerate(blocks):
                    xw_t = xwp.tile([P, TPB * W], bf16, tag="xw")
                    nc.sync.dma_start(xw_t[:], xw_d.ap()[b, :, :])
                    xw_tiles.append(xw_t)
                    mk_t = mkp.tile([P, MAXNSC * P], f8, tag="mk")
                    nc.sync.dma_start(mk_t[:], mk_d.ap()[b, :, :])
                    mk_tiles.append(mk_t)
                    # score' for all 8 tiles of the block in one grouped op
                    xw3 = xw_t[:].rearrange("p (t w) -> p t w", w=W)
                    nc.vector.tensor_reduce(
                        out=sc_t[:, bi * TPB:(bi + 1) * TPB],
                        in_=xw3, axis=Ax.X, op=Alu.add)

                # batched exp of the DVE score columns (per block parity,
                # pairs of adjacent blocks share a strided 3D view)
                sc3 = sc_t[:].rearrange("p (q t) -> p q t", t=2 * TPB)
                e3 = e_t[:].rearrange("p (q k) -> p q k", k=2 * LABW)
                npair = nb // 2
                if npair:
                    n0 = _nsc(0)
                    nc.scalar.activation(
                        out=e3[:, :npair, 0:TPB - n0],
                        in_=sc3[:, :npair, n0:TPB], func=Act.Exp)
                    n1 = _nsc(1)
                    nc.scalar.activation(
                        out=e3[:, :npair, LABW:LABW + TPB - n1],
                        in_=sc3[:, :npair, TPB + n1:2 * TPB], func=Act.Exp)
                if nb % 2:
                    bi = nb - 1
                    n0 = _nsc(bi)
                    nc.scalar.activation(
                        out=e_t[:, bi * LABW:bi * LABW + TPB - n0],
                        in_=sc_t[:, bi * TPB + n0:(bi + 1) * TPB],
                        func=Act.Exp)

                for bi, b in enumerate(blocks):
                    xw_t, mk_t = xw_tiles[bi], mk_tiles[bi]
                    nsc_b = _nsc(bi)
                    ps = psp.tile([P, W], f32, tag="acc")
                    for t in range(TPB):
                        me = mep.tile([P, P], bf16, tag="me")
                        if t < nsc_b:
                            nc.scalar.activation(
                                out=me[:], in_=mk_t[:, t * P:(t + 1) * P],
                                func=Act.Exp,
                                bias=sc_t[:, bi * TPB + t:bi * TPB + t + 1],
                                scale=1.0)
                        else:
                            lcol = b * LABW + (t - nsc_b)
                            ecol = bi * LABW + (t - nsc_b)
                            nc.vector.tensor_scalar(
                                out=me[:], in0=iota_b[:],
                                scalar1=lab_all[:, lcol:lcol + 1],
                                scalar2=e_t[:, ecol:ecol + 1],
                                op0=Alu.is_equal, op1=Alu.mult)
                        nc.tensor.matmul(ps[:], lhsT=me[:],
                                         rhs=xw_t[:, t * W:(t + 1) * W],
                                         start=(t == 0), stop=(t == TPB - 1))
                    ev = evp.tile([P, P + 1], f32, tag="ev")
                    nc.vector.tensor_copy(ev[:], ps[:, 0:P + 1])
                    nc.sync.dma_start(out_d.ap()[b, :, :], ev[:])

    nc.compile()
    return nc


# ---------------------------------------------------------------- host side
def _pack_blocks(counts):
    blocks = []
    s, nseg = 0, len(counts)
    while s < nseg:
        rows, s0 = 0, s
        while s < nseg and s - s0 < MAX_SEGS_PER_BLOCK:
            c = counts[s]
            if rows + c > ROWS_PER_BLOCK:
                break
            rows += int(c)
            s += 1
        assert s > s0, f"segment {s0} with {counts[s0]} rows exceeds a block"
        blocks.append((s0, s, rows))
    return blocks


def _numpy_fallback(x, labels, w, b):
    scores = x.astype(np.float64) @ w.astype(np.float64) + float(b)
    scores -= scores.max()
    e = np.exp(scores)
    a = e / e.sum()
    out = np.zeros((NUM_SEGMENTS, x.shape[1]), np.float64)
    np.add.at(out, labels, x * a[:, None])
    return out.astype(np.float32)


def kernel(x, monomer_labels_i, attn_w, attn_b):
    from concourse import bass_utils

    x = np.ascontiguousarray(np.asarray(x, dtype=np.float32))
    labels = np.asarray(monomer_labels_i).astype(np.int64)
    w = np.asarray(attn_w, dtype=np.float32)
    b = np.float32(np.asarray(attn_b))

    counts = np.bincount(labels, minlength=NUM_SEGMENTS)
    if np.abs(w).min() < 1e-30 or counts.max() > ROWS_PER_BLOCK:
        return _numpy_fallback(x, labels, w, b)

    order = np.argsort(labels, kind="stable")
    labels_s = labels[order]
    blocks = _pack_blocks(counts)
    nblocks = len(blocks)
    B = (nblocks + N_CORES - 1) // N_CORES

    s0_arr = np.fromiter((blk[0] for blk in blocks), np.int64, nblocks)
    s1_arr = np.fromiter((blk[1] for blk in blocks), np.int64, nblocks)
    rows_arr = np.fromiter((blk[2] for blk in blocks), np.int64, nblocks)
    r0_arr = np.zeros(nblocks, np.int64)
    np.cumsum(rows_arr[:-1], out=r0_arr[1:])

    # per-row placement (all rows, sorted order)
    blk_r = np.repeat(np.arange(nblocks), rows_arr)
    pos = np.arange(N) - r0_arr[blk_r]
    tile_r = (pos >> 7).astype(np.int64)
    p_r = (pos & 127).astype(np.int64)
    lab_rel = (labels_s - s0_arr[blk_r]).astype(np.int64)
    core_r = blk_r // B
    bil_r = blk_r - core_r * B
    nsc_arr = np.fromiter((_nsc(i) for i in range(B)), np.int64, B)
    nsc_r = nsc_arr[bil_r]
    on_scale = tile_r < nsc_r

    xw = (x[order] * w[None, :]).astype(ml_dtypes.bfloat16)

    xw_all = np.zeros((N_CORES, B, P, TPB, W), ml_dtypes.bfloat16)
    xw_all[:, :, :, :, 128] = 1.0
    xw_all[core_r, bil_r, p_r, tile_r, 0:128] = xw

    byte_neg = np.array(-96.0, ml_dtypes.float8_e4m3).view(np.uint8)
    mk_all = np.full((N_CORES, B, P, MAXNSC * P), byte_neg, np.uint8)
    m = on_scale
    mk_all[core_r[m], bil_r[m], p_r[m], tile_r[m] * P + lab_rel[m]] = \
        np.array(0.0, ml_dtypes.float8_e4m3).view(np.uint8)

    lab_all = np.full((N_CORES, P, B * LABW), 500.0, np.float32)
    m2 = ~on_scale
    lab_all[core_r[m2], p_r[m2],
            bil_r[m2] * LABW + tile_r[m2] - nsc_r[m2]] = lab_rel[m2]

    in_maps = [{"xw": xw_all[c].reshape(B, P, TPB * W),
                "mk": mk_all[c].view(ml_dtypes.float8_e4m3),
                "lab": lab_all[c]} for c in range(N_CORES)]

    key = (B, NSC_PATTERN)
    if key not in _COMPILED:
        _COMPILED[key] = _build_kernel(B)
    nc = _COMPILED[key]

    res = bass_utils.run_bass_kernel_spmd(nc, in_maps,
                                          core_ids=list(range(N_CORES)))

    # ---- gather / unshard
    Z = 0.0
    out = np.zeros((NUM_SEGMENTS, D), np.float32)
    for c in range(N_CORES):
        od = res.results[c]["out"]          # [B, P, 129]
        Z += float(od[:, :, 128].astype(np.float64).sum())
        for bi in range(B):
            gi = c * B + bi
            if gi >= nblocks:
                continue
            s0, s1 = int(s0_arr[gi]), int(s1_arr[gi])
            out[s0:s1] = od[bi, :s1 - s0, :128]
    out /= (w[None, :].astype(np.float64) * Z)
    return out.astype(np.float32)


if __name__ == "__main__":
    from ref_io import get
    inputs, expected = get()
    out = kernel(**inputs)
    err = np.abs(out - expected)
    print("absmax err:", err.max(), "scale-rel:",
          err.max() / np.abs(expected).max())


# revision 7
# speedup vs baseline: 2.4020x; 1.3160x over previous
"""Trainium2 Bass kernel for AttentionReadoutAtom (global-softmax segment reduce).

Math:  scores = x @ w + b ; attn = softmax(scores over all N) ;
       out[s] = sum_{i: label_i = s} attn_i * x_i          -> [50000, 128]

Softmax is shift/scale invariant, so exp(score) without max-subtraction is
safe (scores ~ N(0,1)) and the bias b cancels.  Using xw = x * w:

    out[s, d] = sum_{i in s} e_i * xw_i[d] / (w[d] * Z),   Z = sum_i e_i

Sharding (host, inside kernel()): sort rows by segment label, greedily pack
whole segments into blocks of 1024 rows (8 row-tiles of 128) covering <=128
distinct segments; blocks are dealt contiguously to 8 cores.  Every segment
lives in exactly one block, so no cross-core combine is needed; the only
global quantity is Z, reduced on the host (the hint's denominator
all-reduce).

Device, per block of 8 row-tiles (Tile framework schedules all engines):
  * score'[p, t] = sum_w xw_aug[p, t, w]  -- ONE grouped DVE tensor_reduce
    (bf16 out, fp32 internal accum -> DVE 2x packed mode).  Column 128 of
    each tile is 1.0 and col 129 is 0.0, so score' = score + 1: a constant
    softmax shift that cancels.
  * me[p, s] = e'_p * onehot(lab_p)[s], built three ways (engine-balanced
    per ATTN_TILE_PATTERN, one char per row-tile):
      s: ScalarE fused ACTIVATE me = Exp(logmask_fp8 + score'): the
         host-shipped fp8 tile is 0 at (p, lab_p), -96 elsewhere; score'
         rides the per-partition bias port.  exp(-96+s) == 0.
      g: GpSimd local_scatter of e' (bf16, from a small batched exp) into a
         zeroed [128, 128] tile at host-shipped int16 (lab, -1) pairs.
      v: DVE dual-ALU tensor_scalar me = (iota == lab) * e'.
  * psum[s, w] += me^T @ xw_aug_tile  (TensorE, PSUM accum over the block).
    Column 128 of the result is the per-segment sum of e' (pad rows have
    me == 0 everywhere so they are excluded) -> Z on host.
  * evict psum -> chunk SBUF tile (DVE copy) -> one DMA per 4 blocks.

DMA issue cost on the SP engine is ~625 ns per DMA, so inputs ship as ONE
uint8 tensor per 4-block chunk (xw_aug bytes + fp8 mask bytes per block,
bitcast on SBUF into bf16 / fp8 views): 3 DMAs per chunk minus outputs.

Host epilogue: Z = sum of column-128, scatter block rows to the full
output, divide by w[d] * Z.
"""

import os
import numpy as np
import ml_dtypes

# ---------------------------------------------------------------- constants
N = 500000
D = 128
NUM_SEGMENTS = 50000
N_CORES = 8
P = 128
TPB = 8                   # row tiles per block
ROWS_PER_BLOCK = TPB * P  # 1024
MAX_SEGS_PER_BLOCK = 128
W = 130                   # cols per tile in xw_aug: 128 xw + 1.0 + 0.0
CB = 4                    # blocks per chunk (one input DMA, one output DMA)

# per-row-tile engine assignment: s=ScalarE fused, g=GpSimd scatter, v=DVE
PATTERN = os.environ.get("ATTN_TILE_PATTERN", "ssssgggv")
assert len(PATTERN) == TPB and set(PATTERN) <= set("sgv")
NS = PATTERN.count("s")
NG = PATTERN.count("g")
NV = PATTERN.count("v")
K = NG + NV               # tiles needing explicit e'
BPB = TPB * W * 2 + NS * P  # bytes per block in the combined input tensor

_COMPILED = {}


# ---------------------------------------------------------------- device code
def _build_kernel(B):
    import concourse.bacc as bacc
    import concourse.mybir as mybir
    from concourse.tile import TileContext
    from concourse import library_config

    f32 = mybir.dt.float32
    bf16 = mybir.dt.bfloat16
    f8 = mybir.dt.float8e4
    i16 = mybir.dt.int16
    f16 = mybir.dt.float16
    u8 = mybir.dt.uint8
    Alu = mybir.AluOpType
    Act = mybir.ActivationFunctionType
    Ax = mybir.AxisListType

    # tile index (within block) of the k-th 's'/'g'/'v' slot
    s_tiles = [t for t, c in enumerate(PATTERN) if c == "s"]
    g_tiles = [t for t, c in enumerate(PATTERN) if c == "g"]
    v_tiles = [t for t, c in enumerate(PATTERN) if c == "v"]
    # e' column (within the K-wide per-block e group) for each g/v tile
    e_slot = {t: j for j, t in enumerate(g_tiles + v_tiles)}

    NCHUNK = (B + CB - 1) // CB

    nc = bacc.Bacc("TRN2", target_bir_lowering=False, debug=False,
                   num_devices=N_CORES)

    xm_d = nc.dram_tensor("xm", [NCHUNK, P, CB * BPB], u8,
                          kind="ExternalInput")
    labi_d = nc.dram_tensor("labi", [P, max(1, B * NG) * 2], i16,
                            kind="ExternalInput")
    labf_d = nc.dram_tensor("labf", [P, max(1, B * NV)], f32,
                            kind="ExternalInput")
    out_d = nc.dram_tensor("out", [NCHUNK, P, CB * (P + 1)], f32,
                           kind="ExternalOutput")

    with TileContext(nc) as tc:
        with tc.tile_pool(name="const", bufs=1) as cpool, \
             tc.tile_pool(name="xmp", bufs=3) as xmp, \
             tc.tile_pool(name="scp", bufs=3) as scp, \
             tc.tile_pool(name="mep", bufs=12) as mep, \
             tc.tile_pool(name="evp", bufs=3) as evp, \
             tc.tile_pool(name="psum", bufs=8, space="PSUM") as psp:

            iota_i = cpool.tile([P, P], mybir.dt.int32)
            nc.gpsimd.iota(iota_i[:], pattern=[[1, P]], base=0,
                           channel_multiplier=0)
            iota_b = cpool.tile([P, P], bf16)
            nc.vector.tensor_copy(iota_b[:], iota_i[:])
            if NG:
                nc.gpsimd.load_library(library_config.local_scatter)

            labi = cpool.tile([P, max(1, B * NG) * 2], i16)
            nc.sync.dma_start(labi[:], labi_d.ap()[:, :])
            labf = cpool.tile([P, max(1, B * NV)], f32)
            nc.sync.dma_start(labf[:], labf_d.ap()[:, :])

            for ch in range(NCHUNK):
                blocks = list(range(ch * CB, min((ch + 1) * CB, B)))
                nb = len(blocks)

                xm_t = xmp.tile([P, CB * BPB], u8, tag="xm")
                nc.sync.dma_start(xm_t[:, :nb * BPB],
                                  xm_d.ap()[ch, :, :nb * BPB])

                sc_t = scp.tile([P, nb * TPB], f16, tag="sc")
                eg_t = scp.tile([P, max(1, nb * NG) + 2], bf16, tag="eg")
                ev_t = scp.tile([P, max(1, nb * NV)], f32, tag="ev")

                xw_views, mk_views = [], []
                for bi in range(nb):
                    blk = xm_t[:, bi * BPB:(bi + 1) * BPB]
                    xw_v = blk[:, :TPB * W * 2].bitcast(bf16)
                    mk_v = blk[:, TPB * W * 2:].bitcast(f8)
                    xw_views.append(xw_v)
                    mk_views.append(mk_v)
                    with nc.allow_low_precision(
                            reason="score accum is fp32 internal; bf16 "
                                   "rounding of the final sum only"):
                        nc.vector.tensor_reduce(
                            out=sc_t[:, bi * TPB:(bi + 1) * TPB],
                            in_=xw_v.rearrange("p (t w) -> p t w", w=W),
                            axis=Ax.X, op=Alu.add)

                sc3 = sc_t[:].rearrange("p (b t) -> p b t", t=TPB)
                for tl, et, nk in ((g_tiles, eg_t, NG), (v_tiles, ev_t, NV)):
                    if not nk:
                        continue
                    e3 = et[:, :nb * nk].rearrange("p (b k) -> p b k", k=nk)
                    r0 = 0
                    for j in range(1, nk + 1):
                        if j == nk or tl[j] != tl[j - 1] + 1:
                            nc.scalar.activation(
                                out=e3[:, :, r0:j],
                                in_=sc3[:, :, tl[r0]:tl[r0] + (j - r0)],
                                func=Act.Exp)
                            r0 = j

                ev = evp.tile([P, CB * (P + 1)], f32, tag="ev")
                for bi, b in enumerate(blocks):
                    xw_v, mk_v = xw_views[bi], mk_views[bi]
                    ps = psp.tile([P, W], f32, tag="acc")
                    si = 0
                    for t in range(TPB):
                        me = mep.tile([P, P], bf16, tag="me")
                        if PATTERN[t] == "s":
                            nc.scalar.activation(
                                out=me[:], in_=mk_v[:, si * P:(si + 1) * P],
                                func=Act.Exp,
                                bias=sc_t[:, bi * TPB + t:bi * TPB + t + 1],
                                scale=1.0)
                            si += 1
                        elif PATTERN[t] == "g":
                            lcol = (b * NG + g_tiles.index(t)) * 2
                            ecol = bi * NG + g_tiles.index(t)
                            nc.gpsimd.local_scatter(
                                me[:], eg_t[:, ecol:ecol + 2],
                                labi[:, lcol:lcol + 2],
                                channels=P, num_elems=P, num_idxs=2)
                        else:
                            lcol = b * NV + v_tiles.index(t)
                            ecol = bi * NV + v_tiles.index(t)
                            nc.vector.tensor_scalar(
                                out=me[:], in0=iota_b[:],
                                scalar1=labf[:, lcol:lcol + 1],
                                scalar2=ev_t[:, ecol:ecol + 1],
                                op0=Alu.is_equal, op1=Alu.mult)
                        nc.tensor.matmul(ps[:], lhsT=me[:],
                                         rhs=xw_v[:, t * W:(t + 1) * W],
                                         start=(t == 0), stop=(t == TPB - 1))
                    nc.vector.tensor_copy(
                        ev[:, bi * (P + 1):(bi + 1) * (P + 1)],
                        ps[:, 0:P + 1])
                nc.sync.dma_start(out_d.ap()[ch, :, :nb * (P + 1)],
                                  ev[:, :nb * (P + 1)])

    nc.compile()
    return nc


# ---------------------------------------------------------------- host side
def _pack_blocks(counts):
    blocks = []
    s, nseg = 0, len(counts)
    while s < nseg:
        rows, s0 = 0, s
        while s < nseg and s - s0 < MAX_SEGS_PER_BLOCK:
            c = counts[s]
            if rows + c > ROWS_PER_BLOCK:
                break
            rows += int(c)
            s += 1
        assert s > s0, f"segment {s0} with {counts[s0]} rows exceeds a block"
        blocks.append((s0, s, rows))
    return blocks


def _numpy_fallback(x, labels, w, b):
    scores = x.astype(np.float64) @ w.astype(np.float64) + float(b)
    scores -= scores.max()
    e = np.exp(scores)
    a = e / e.sum()
    out = np.zeros((NUM_SEGMENTS, x.shape[1]), np.float64)
    np.add.at(out, labels, x * a[:, None])
    return out.astype(np.float32)


def kernel(x, monomer_labels_i, attn_w, attn_b):
    from concourse import bass_utils

    x = np.ascontiguousarray(np.asarray(x, dtype=np.float32))
    labels = np.asarray(monomer_labels_i).astype(np.int64)
    w = np.asarray(attn_w, dtype=np.float32)
    b = np.float32(np.asarray(attn_b))

    counts = np.bincount(labels, minlength=NUM_SEGMENTS)
    if np.abs(w).min() < 1e-30 or counts.max() > ROWS_PER_BLOCK:
        return _numpy_fallback(x, labels, w, b)

    order = np.argsort(labels, kind="stable")
    labels_s = labels[order]
    blocks = _pack_blocks(counts)
    nblocks = len(blocks)
    B = (nblocks + N_CORES - 1) // N_CORES
    NCHUNK = (B + CB - 1) // CB

    s0_arr = np.fromiter((blk[0] for blk in blocks), np.int64, nblocks)
    s1_arr = np.fromiter((blk[1] for blk in blocks), np.int64, nblocks)
    rows_arr = np.fromiter((blk[2] for blk in blocks), np.int64, nblocks)
    r0_arr = np.zeros(nblocks, np.int64)
    np.cumsum(rows_arr[:-1], out=r0_arr[1:])

    # per-row placement (all rows, sorted order)
    blk_r = np.repeat(np.arange(nblocks), rows_arr)
    pos = np.arange(N) - r0_arr[blk_r]
    tile_r = (pos >> 7).astype(np.int64)
    p_r = (pos & 127).astype(np.int64)
    lab_rel = (labels_s - s0_arr[blk_r]).astype(np.int64)
    core_r = blk_r // B
    bil_r = blk_r - core_r * B

    s_tiles = [t for t, c in enumerate(PATTERN) if c == "s"]
    g_tiles = [t for t, c in enumerate(PATTERN) if c == "g"]
    v_tiles = [t for t, c in enumerate(PATTERN) if c == "v"]
    # map tile index -> slot index within its class (-1 if other class)
    s_slot = np.full(TPB, -1, np.int64)
    g_slot = np.full(TPB, -1, np.int64)
    v_slot = np.full(TPB, -1, np.int64)
    for j, t in enumerate(s_tiles):
        s_slot[t] = j
    for j, t in enumerate(g_tiles):
        g_slot[t] = j
    for j, t in enumerate(v_tiles):
        v_slot[t] = j

    xw = (x[order] * w[None, :]).astype(ml_dtypes.bfloat16)

    # combined xw_aug + mask bytes, [cores, B, P, BPB] u8
    xw_blk = np.zeros((N_CORES, B, P, TPB, W), ml_dtypes.bfloat16)
    xw_blk[:, :, :, :, 128] = 1.0
    xw_blk[core_r, bil_r, p_r, tile_r, 0:128] = xw

    byte_neg = np.array(-96.0, ml_dtypes.float8_e4m3).view(np.uint8)
    mk_blk = np.full((N_CORES, B, P, NS * P), byte_neg, np.uint8)
    m = s_slot[tile_r] >= 0
    mk_blk[core_r[m], bil_r[m], p_r[m],
           s_slot[tile_r[m]] * P + lab_rel[m]] = \
        np.array(0.0, ml_dtypes.float8_e4m3).view(np.uint8)

    xm_all = np.concatenate(
        [xw_blk.reshape(N_CORES, B, P, TPB * W).view(np.uint8)
         .reshape(N_CORES, B, P, TPB * W * 2),
         mk_blk], axis=3)                       # [cores, B, P, BPB]
    pad_blocks = NCHUNK * CB - B
    if pad_blocks:
        xm_all = np.concatenate(
            [xm_all, np.zeros((N_CORES, pad_blocks, P, BPB), np.uint8)],
            axis=1)
    xm_all = (xm_all.reshape(N_CORES, NCHUNK, CB, P, BPB)
              .transpose(0, 1, 3, 2, 4)
              .reshape(N_CORES, NCHUNK, P, CB * BPB))
    xm_all = np.ascontiguousarray(xm_all)

    labi_all = np.full((N_CORES, P, max(1, B * NG), 2), -1, np.int16)
    if NG:
        m = g_slot[tile_r] >= 0
        labi_all[core_r[m], p_r[m],
                 bil_r[m] * NG + g_slot[tile_r[m]], 0] = lab_rel[m]
    labf_all = np.full((N_CORES, P, max(1, B * NV)), 500.0, np.float32)
    if NV:
        m = v_slot[tile_r] >= 0
        labf_all[core_r[m], p_r[m],
                 bil_r[m] * NV + v_slot[tile_r[m]]] = lab_rel[m]

    in_maps = [{"xm": xm_all[c],
                "labi": labi_all[c].reshape(P, -1),
                "labf": labf_all[c]} for c in range(N_CORES)]

    key = (B, PATTERN)
    if key not in _COMPILED:
        _COMPILED[key] = _build_kernel(B)
    nc = _COMPILED[key]

    res = bass_utils.run_bass_kernel_spmd(nc, in_maps,
                                          core_ids=list(range(N_CORES)))

    # ---- gather / unshard
    Z = 0.0
    out = np.zeros((NUM_SEGMENTS, D), np.float32)
    for c in range(N_CORES):
        od = res.results[c]["out"].reshape(NCHUNK, P, CB, P + 1)
        Z += float(od[:, :, :, 128].astype(np.float64).sum())
        od = od.transpose(0, 2, 1, 3)           # [NCHUNK, CB, P, 129]
        for bi in range(B):
            gi = c * B + bi
            if gi >= nblocks:
                continue
            s0, s1 = int(s0_arr[gi]), int(s1_arr[gi])
            out[s0:s1] = od[bi // CB, bi % CB, :s1 - s0, :128]
    out /= (w[None, :].astype(np.float64) * Z)
    return out.astype(np.float32)


if __name__ == "__main__":
    from ref_io import get
    inputs, expected = get()
    out = kernel(**inputs)
    err = np.abs(out - expected)
    print("absmax err:", err.max(), "scale-rel:",
          err.max() / np.abs(expected).max())


# revision 16
# speedup vs baseline: 2.8874x; 1.2021x over previous
"""Trainium2 Bass kernel for AttentionReadoutAtom (global-softmax segment reduce).

Math:  scores = x @ w + b ; attn = softmax(scores over all N) ;
       out[s] = sum_{i: label_i = s} attn_i * x_i          -> [50000, 128]

Softmax is shift/scale invariant, so exp(score) without max-subtraction is
safe (scores ~ N(0,1)) and the bias b cancels.  Using xw = x * w:

    out[s, d] = sum_{i in s} e_i * xw_i[d] / (w[d] * Z),   Z = sum_i e_i

Sharding (host, inside kernel()): sort rows by segment label, greedily pack
whole segments into blocks of 1024 rows (8 row-tiles of 128) covering <=128
distinct segments; blocks are dealt contiguously to 8 cores.  Every segment
lives in exactly one block, so no cross-core combine is needed; the only
global quantity is Z, reduced on the host (the hint's denominator
all-reduce).

Device, per block of 8 row-tiles (Tile framework schedules all engines):
  * score'[p, t]: one 4x-mode DVE tensor_scalar with accum_out per tile
    (immediate scalars keep the fast mode; fp16 out), or one grouped 1x
    tensor_reduce per chunk (ATTN_SCORE).  Column 128 of each tile is 1.0
    and col 129 is 0.0, so score' = score + 1: a constant softmax shift
    that cancels in the normalization.
  * me[p, s] = e'_p * onehot(lab_p)[s], engine-split per ATTN_TILE_PATTERN
    (one char per row-tile):
      s: ScalarE fused ACTIVATE me = Exp(logmask_fp8 + score'): the
         host-shipped fp8 tile is 0 at (p, lab_p), -96 elsewhere; score'
         rides the per-partition bias port.  exp(-96+s) == 0.
      g: ONE GpSimd local_scatter per block builds ALL g tiles: e' values
         (bf16, from a batched exp) scatter into a zeroed [128, G*128]
         tile at host-shipped int16 indices tile*128 + lab (pads: -1,
         ignored; real tiles form a prefix per partition so negatives
         always trail).
      v: DVE dual-ALU tensor_scalar me = (iota == lab) * e'.
  * psum[s, w] += me^T @ xw_aug_tile  (TensorE; two blocks share one PSUM
    bank).  Column 128 of the result is the per-segment sum of e' (pad
    rows have me == 0 everywhere) -> Z on host.
  * evict 2-block psum -> chunk SBUF tile (DVE or ScalarE copy,
    ATTN_EVICT) -> one DMA per 4-block chunk.

DMA issue cost on the SP engine is ~625 ns per DMA, so inputs ship as ONE
uint8 tensor per 4-block chunk (xw_aug bytes + fp8 mask bytes per block,
bitcast on SBUF into bf16 / fp8 views).

Host epilogue: Z = sum of column-128, scatter block rows to the full
output, divide by w[d] * Z.
"""

import os
import numpy as np
import ml_dtypes

# ---------------------------------------------------------------- constants
N = 500000
D = 128
NUM_SEGMENTS = 50000
N_CORES = 8
P = 128
TPB = 8                   # row tiles per block
ROWS_PER_BLOCK = TPB * P  # 1024
MAX_SEGS_PER_BLOCK = 128
W = 130                   # cols per tile in xw_aug: 128 xw + 1.0 + 0.0
CB = 4                    # blocks per chunk (one input DMA, one output DMA)

# per-row-tile engine assignment: s=ScalarE fused, g=GpSimd scatter, v=DVE
PATTERN = os.environ.get("ATTN_TILE_PATTERN", "sssggggg")
assert len(PATTERN) == TPB and set(PATTERN) <= set("sgv")
NS = PATTERN.count("s")
NG = PATTERN.count("g")
NV = PATTERN.count("v")
IDW = ((NG + 1 + 3) // 4) * 4 if NG else 0  # num_idxs, 16B-aligned slices
EGW = 8                   # e' slots per block (16B-aligned data slices)
SCORE = os.environ.get("ATTN_SCORE", "ts4x")  # "ts4x" | "reduce"
EVICT = os.environ.get("ATTN_EVICT", "dve")   # "dve" | "act"
BPB = TPB * W * 2 + NS * P  # bytes per block in the combined input tensor

_COMPILED = {}


# ---------------------------------------------------------------- device code
def _build_kernel(B):
    import concourse.bacc as bacc
    import concourse.mybir as mybir
    from concourse.tile import TileContext
    from concourse import library_config

    f32 = mybir.dt.float32
    bf16 = mybir.dt.bfloat16
    f8 = mybir.dt.float8e4
    i16 = mybir.dt.int16
    f16 = mybir.dt.float16
    u8 = mybir.dt.uint8
    Alu = mybir.AluOpType
    Act = mybir.ActivationFunctionType
    Ax = mybir.AxisListType

    s_tiles = [t for t, c in enumerate(PATTERN) if c == "s"]
    g_tiles = [t for t, c in enumerate(PATTERN) if c == "g"]
    v_tiles = [t for t, c in enumerate(PATTERN) if c == "v"]
    if g_tiles:
        assert g_tiles == list(range(g_tiles[0], g_tiles[0] + NG)), \
            "g tiles must be contiguous for the grouped scatter"

    NCHUNK = (B + CB - 1) // CB

    nc = bacc.Bacc("TRN2", target_bir_lowering=False, debug=False,
                   num_devices=N_CORES)

    xm_d = nc.dram_tensor("xm", [NCHUNK, P, CB * BPB], u8,
                          kind="ExternalInput")
    labi_d = nc.dram_tensor("labi", [P, max(1, B * IDW)], i16,
                            kind="ExternalInput")
    labf_d = nc.dram_tensor("labf", [P, max(1, B * NV)], f32,
                            kind="ExternalInput")
    out_d = nc.dram_tensor("out", [NCHUNK, P, CB * W], f32,
                           kind="ExternalOutput")
    dbg_d = None
    if os.environ.get("ATTN_DEBUG_MEG"):
        dbg_d = nc.dram_tensor("dbg", [B, P, NG * P], mybir.dt.bfloat16,
                               kind="ExternalOutput")

    with TileContext(nc) as tc:
        with tc.tile_pool(name="const", bufs=1) as cpool, \
             tc.tile_pool(name="xmp", bufs=3) as xmp, \
             tc.tile_pool(name="scp", bufs=3) as scp, \
             tc.tile_pool(name="mep", bufs=8) as mep, \
             tc.tile_pool(name="mgp", bufs=4) as mgp, \
             tc.tile_pool(name="evp", bufs=3) as evp, \
             tc.tile_pool(name="psum", bufs=4, space="PSUM") as psp:

            iota_b = None
            if NV:
                iota_i = cpool.tile([P, P], mybir.dt.int32)
                nc.gpsimd.iota(iota_i[:], pattern=[[1, P]], base=0,
                               channel_multiplier=0)
                iota_b = cpool.tile([P, P], bf16)
                nc.vector.tensor_copy(iota_b[:], iota_i[:])
            if NG:
                nc.gpsimd.load_library(library_config.local_scatter)

            labi = cpool.tile([P, max(1, B * IDW)], i16)
            nc.sync.dma_start(labi[:], labi_d.ap()[:, :])
            labf = cpool.tile([P, max(1, B * NV)], f32)
            nc.sync.dma_start(labf[:], labf_d.ap()[:, :])

            for ch in range(NCHUNK):
                blocks = list(range(ch * CB, min((ch + 1) * CB, B)))
                nb = len(blocks)

                xm_t = xmp.tile([P, CB * BPB], u8, tag="xm")
                nc.sync.dma_start(xm_t[:, :nb * BPB],
                                  xm_d.ap()[ch, :, :nb * BPB])

                sc_t = scp.tile([P, nb * TPB], f16, tag="sc")
                eg_t = scp.tile([P, max(1, nb * EGW) + IDW], bf16, tag="eg")
                ev_t = scp.tile([P, max(1, nb * NV)], f32, tag="ev")
                junk = scp.tile([P, W], bf16, tag="junk")

                xw_views, mk_views = [], []
                for bi in range(nb):
                    blk = xm_t[:, bi * BPB:(bi + 1) * BPB]
                    xw_views.append(blk[:, :TPB * W * 2].bitcast(bf16))
                    mk_views.append(blk[:, TPB * W * 2:].bitcast(f8))

                if SCORE == "ts4x":
                    for bi in range(nb):
                        for t in range(TPB):
                            with nc.allow_low_precision(
                                    reason="fp32 internal accum"):
                                nc.vector.tensor_scalar(
                                    out=junk[:],
                                    in0=xw_views[bi][:, t * W:(t + 1) * W],
                                    scalar1=1.0, scalar2=0.0,
                                    op0=Alu.mult, op1=Alu.add,
                                    accum_out=sc_t[:, bi * TPB + t:
                                                   bi * TPB + t + 1])
                else:
                    xw_all = (xm_t[:, :nb * BPB].bitcast(bf16)
                              .rearrange("p (b z) -> p b z", z=BPB // 2)
                              [:, :, :TPB * W]
                              .rearrange("p b (t w) -> p b t w", w=W))
                    with nc.allow_low_precision(
                            reason="fp32 internal accum"):
                        nc.vector.tensor_reduce(
                            out=sc_t[:], in_=xw_all, axis=Ax.X, op=Alu.add)

                sc3 = sc_t[:].rearrange("p (b t) -> p b t", t=TPB)
                for tl, et, nk, kw in ((g_tiles, eg_t, NG, EGW),
                                       (v_tiles, ev_t, NV, NV)):
                    if not nk:
                        continue
                    e3 = et[:, :nb * kw].rearrange("p (b k) -> p b k",
                                                   k=kw)[:, :, :nk]
                    r0 = 0
                    for j in range(1, nk + 1):
                        if j == nk or tl[j] != tl[j - 1] + 1:
                            nc.scalar.activation(
                                out=e3[:, :, r0:j],
                                in_=sc3[:, :, tl[r0]:tl[r0] + (j - r0)],
                                func=Act.Exp)
                            r0 = j

                ev = evp.tile([P, CB * W], f32, tag="evc")
                for bi, b in enumerate(blocks):
                    xw_v, mk_v = xw_views[bi], mk_views[bi]
                    if bi % 2 == 0:
                        # full PSUM bank: both 130-col halves stay inside
                        # one bank (matmul accumulation cannot cross banks)
                        ps2 = psp.tile([P, 512], f32, tag="acc")
                    ps = ps2[:, (bi % 2) * W:(bi % 2) * W + W]
                    me_g = None
                    if NG:
                        me_g = mgp.tile([P, NG * P], bf16, tag="meg")
                        nc.gpsimd.local_scatter(
                            me_g[:], eg_t[:, bi * EGW:bi * EGW + IDW],
                            labi[:, b * IDW:(b + 1) * IDW],
                            channels=P, num_elems=NG * P, num_idxs=IDW)
                    if dbg_d is not None and NG:
                        nc.sync.dma_start(dbg_d.ap()[b, :, :], me_g[:])
                    si = 0
                    for t in range(TPB):
                        if PATTERN[t] == "s":
                            me = mep.tile([P, P], bf16, tag="me")
                            nc.scalar.activation(
                                out=me[:], in_=mk_v[:, si * P:(si + 1) * P],
                                func=Act.Exp,
                                bias=sc_t[:, bi * TPB + t:bi * TPB + t + 1],
                                scale=1.0)
                            si += 1
                            lhs = me[:]
                        elif PATTERN[t] == "g":
                            j = g_tiles.index(t)
                            lhs = me_g[:, j * P:(j + 1) * P]
                        else:
                            me = mep.tile([P, P], bf16, tag="me")
                            lcol = b * NV + v_tiles.index(t)
                            ecol = bi * NV + v_tiles.index(t)
                            nc.vector.tensor_scalar(
                                out=me[:], in0=iota_b[:],
                                scalar1=labf[:, lcol:lcol + 1],
                                scalar2=ev_t[:, ecol:ecol + 1],
                                op0=Alu.is_equal, op1=Alu.mult)
                            lhs = me[:]
                        nc.tensor.matmul(ps, lhsT=lhs,
                                         rhs=xw_v[:, t * W:(t + 1) * W],
                                         start=(t == 0), stop=(t == TPB - 1))
                    if bi % 2 == 1 or bi == nb - 1:
                        npsb = (bi % 2) + 1
                        lo = (bi - npsb + 1) * W
                        if EVICT == "dve":
                            nc.vector.tensor_copy(
                                ev[:, lo:lo + npsb * W],
                                ps2[:, :npsb * W])
                        else:
                            nc.scalar.copy(ev[:, lo:lo + npsb * W],
                                           ps2[:, :npsb * W])
                nc.sync.dma_start(out_d.ap()[ch, :, :nb * W],
                                  ev[:, :nb * W])

    nc.compile()
    return nc


# ---------------------------------------------------------------- host side
def _pack_blocks(counts):
    blocks = []
    s, nseg = 0, len(counts)
    while s < nseg:
        rows, s0 = 0, s
        while s < nseg and s - s0 < MAX_SEGS_PER_BLOCK:
            c = counts[s]
            if rows + c > ROWS_PER_BLOCK:
                break
            rows += int(c)
            s += 1
        assert s > s0, f"segment {s0} with {counts[s0]} rows exceeds a block"
        blocks.append((s0, s, rows))
    return blocks


def _numpy_fallback(x, labels, w, b):
    scores = x.astype(np.float64) @ w.astype(np.float64) + float(b)
    scores -= scores.max()
    e = np.exp(scores)
    a = e / e.sum()
    out = np.zeros((NUM_SEGMENTS, x.shape[1]), np.float64)
    np.add.at(out, labels, x * a[:, None])
    return out.astype(np.float32)


def kernel(x, monomer_labels_i, attn_w, attn_b):
    from concourse import bass_utils

    x = np.ascontiguousarray(np.asarray(x, dtype=np.float32))
    labels = np.asarray(monomer_labels_i).astype(np.int64)
    w = np.asarray(attn_w, dtype=np.float32)
    b = np.float32(np.asarray(attn_b))

    counts = np.bincount(labels, minlength=NUM_SEGMENTS)
    if np.abs(w).min() < 1e-30 or counts.max() > ROWS_PER_BLOCK:
        return _numpy_fallback(x, labels, w, b)

    order = np.argsort(labels, kind="stable")
    labels_s = labels[order]
    blocks = _pack_blocks(counts)
    nblocks = len(blocks)
    B = (nblocks + N_CORES - 1) // N_CORES
    NCHUNK = (B + CB - 1) // CB

    s0_arr = np.fromiter((blk[0] for blk in blocks), np.int64, nblocks)
    s1_arr = np.fromiter((blk[1] for blk in blocks), np.int64, nblocks)
    rows_arr = np.fromiter((blk[2] for blk in blocks), np.int64, nblocks)
    r0_arr = np.zeros(nblocks, np.int64)
    np.cumsum(rows_arr[:-1], out=r0_arr[1:])

    # per-row placement (all rows, sorted order)
    blk_r = np.repeat(np.arange(nblocks), rows_arr)
    pos = np.arange(N) - r0_arr[blk_r]
    tile_r = (pos >> 7).astype(np.int64)
    p_r = (pos & 127).astype(np.int64)
    lab_rel = (labels_s - s0_arr[blk_r]).astype(np.int64)
    core_r = blk_r // B
    bil_r = blk_r - core_r * B

    s_tiles = [t for t, c in enumerate(PATTERN) if c == "s"]
    g_tiles = [t for t, c in enumerate(PATTERN) if c == "g"]
    v_tiles = [t for t, c in enumerate(PATTERN) if c == "v"]
    s_slot = np.full(TPB, -1, np.int64)
    g_slot = np.full(TPB, -1, np.int64)
    v_slot = np.full(TPB, -1, np.int64)
    for j, t in enumerate(s_tiles):
        s_slot[t] = j
    for j, t in enumerate(g_tiles):
        g_slot[t] = j
    for j, t in enumerate(v_tiles):
        v_slot[t] = j

    xw = (x[order] * w[None, :]).astype(ml_dtypes.bfloat16)

    # combined xw_aug + mask bytes, [cores, B, P, BPB] u8
    xw_blk = np.zeros((N_CORES, B, P, TPB, W), ml_dtypes.bfloat16)
    xw_blk[:, :, :, :, 128] = 1.0
    xw_blk[core_r, bil_r, p_r, tile_r, 0:128] = xw

    byte_neg = np.array(-96.0, ml_dtypes.float8_e4m3).view(np.uint8)
    mk_blk = np.full((N_CORES, B, P, NS * P), byte_neg, np.uint8)
    m = s_slot[tile_r] >= 0
    mk_blk[core_r[m], bil_r[m], p_r[m],
           s_slot[tile_r[m]] * P + lab_rel[m]] = \
        np.array(0.0, ml_dtypes.float8_e4m3).view(np.uint8)

    xm_all = np.concatenate(
        [xw_blk.reshape(N_CORES, B, P, TPB * W).view(np.uint8)
         .reshape(N_CORES, B, P, TPB * W * 2),
         mk_blk], axis=3)                       # [cores, B, P, BPB]
    pad_blocks = NCHUNK * CB - B
    if pad_blocks:
        xm_all = np.concatenate(
            [xm_all, np.zeros((N_CORES, pad_blocks, P, BPB), np.uint8)],
            axis=1)
    xm_all = (xm_all.reshape(N_CORES, NCHUNK, CB, P, BPB)
              .transpose(0, 1, 3, 2, 4)
              .reshape(N_CORES, NCHUNK, P, CB * BPB))
    xm_all = np.ascontiguousarray(xm_all)

    labi_all = np.full((N_CORES, P, max(1, B * IDW)), -1, np.int16)
    if NG:
        m = g_slot[tile_r] >= 0
        labi_all[core_r[m], p_r[m],
                 bil_r[m] * IDW + g_slot[tile_r[m]]] = \
            (g_slot[tile_r[m]] * P + lab_rel[m]).astype(np.int16)
    labf_all = np.full((N_CORES, P, max(1, B * NV)), 500.0, np.float32)
    if NV:
        m = v_slot[tile_r] >= 0
        labf_all[core_r[m], p_r[m],
                 bil_r[m] * NV + v_slot[tile_r[m]]] = lab_rel[m]

    in_maps = [{"xm": xm_all[c],
                "labi": labi_all[c],
                "labf": labf_all[c]} for c in range(N_CORES)]

    key = (B, PATTERN, SCORE, EVICT)
    if key not in _COMPILED:
        _COMPILED[key] = _build_kernel(B)
    nc = _COMPILED[key]

    res = bass_utils.run_bass_kernel_spmd(nc, in_maps,
                                          core_ids=list(range(N_CORES)))

    # ---- gather / unshard
    Z = 0.0
    out = np.zeros((NUM_SEGMENTS, D), np.float32)
    for c in range(N_CORES):
        od = res.results[c]["out"].reshape(NCHUNK, P, CB, W)
        Z += float(od[:, :, :, 128].astype(np.float64).sum())
        od = od.transpose(0, 2, 1, 3)           # [NCHUNK, CB, P, W]
        for bi in range(B):
            gi = c * B + bi
            if gi >= nblocks:
                continue
            s0, s1 = int(s0_arr[gi]), int(s1_arr[gi])
            out[s0:s1] = od[bi // CB, bi % CB, :s1 - s0, :128]
    out /= (w[None, :].astype(np.float64) * Z)
    return out.astype(np.float32)


if __name__ == "__main__":
    from ref_io import get
    inputs, expected = get()
    out = kernel(**inputs)
    err = np.abs(out - expected)
    print("absmax err:", err.max(), "scale-rel:",
          err.max() / np.abs(expected).max())
